# Initial kernel scaffold
#
import sys
for _p in ('/opt/trn_rl_repo',):
    if _p not in sys.path:
        sys.path.insert(0, _p)

"""NLSGCRN cell Bass/Tile kernel for TRN2, batch-sharded SPMD over 8 cores.

Per-core shapes (b_loc = 4 batches):
  x [4,2000,32], state [4,2000,64], x_full [4,12,2000,48], emb [2000,16],
  pools gw/uw/gb/ub/gT/uT, out h [4,2000,64].

v2 structure (vs v1 baseline):
- y-phases loop nch-outer / b-inner: the per-node d-reduction batches all
  4 batches into one op (the e[n,d] scalar is per-partition and shared
  across b), split DVE (d < D_DVE, seeded with bias) / GPSIMD (rest).
- XkT stationaries via PE transposes (identity matmul) into bf16 PSUM,
  evicted by DVE, replacing v1's DRAM-bounce DMA transposes.
- A is spilled to DRAM at generation and its SBUF tile is freed after the
  gate hops; update diffusion streams A chunks back mi-outer with all 16
  output-chunk accumulators resident in PSUM (8 banks exactly).
- PSUM bank budget in y phases: y tiles [128,1024] f32 (2 banks) x2 bufs
  + transpose batch [128,1536] bf16 (2 banks) x2 bufs.
"""

from contextlib import ExitStack

import concourse.bass as bass
import concourse.tile as tile
from concourse import mybir
from concourse._compat import with_exitstack
from concourse.masks import make_identity

F32 = mybir.dt.float32
F32R = mybir.dt.float32r
BF16 = mybir.dt.bfloat16
AF = mybir.ActivationFunctionType
OP = mybir.AluOpType

B_LOC = 4
N = 2000
NCHUNK = 16           # ceil(2000/128)
NFULL = (NCHUNK - 1) * 128   # 1920
NPAD = NCHUNK * 128   # 2048
DIN, DOUT = 32, 64
CIN = 96
CW = 48
WLEN = 12
EMB = 16
K = 3

D_DVE = 8   # d-reduction: first D_DVE iterations on DVE, rest on GPSIMD


def nlen(nch):
    return 128 if nch < NCHUNK - 1 else N - NFULL  # last = 80


def chunked_load(nc, dst, src, eng=None):
    """dst [128, NCHUNK, ...inner] <- src [2000, ...inner] splitting rows."""
    eng = eng or nc.sync
    inner = src.shape[1:]
    eng.dma_start(
        dst[:, 0 : NCHUNK - 1],
        src[0:NFULL].rearrange(
            "(c p) " + " ".join(f"i{j}" for j in range(len(inner)))
            + " -> p c " + " ".join(f"i{j}" for j in range(len(inner))),
            p=128,
        ),
    )
    eng.dma_start(dst[0 : N - NFULL, NCHUNK - 1], src[NFULL:N])


@with_exitstack
def build(ctx: ExitStack, tc: tile.TileContext, io: dict):
    nc = tc.nc

    io = {k: (v[:] if not isinstance(v, bass.AP) else v) for k, v in io.items()}
    x, state, x_full = io["x"], io["state"], io["x_full"]
    emb = io["node_embeddings"]
    out = io["out"]

    const = ctx.enter_context(tc.tile_pool(name="const", bufs=1))
    big = ctx.enter_context(tc.tile_pool(name="big", bufs=1))
    dram = ctx.enter_context(tc.tile_pool(name="dram", bufs=6, space="DRAM"))

    # ================= constants / weights =================
    eexpf = const.tile([128, NCHUNK, EMB], F32)
    nc.vector.memset(eexpf[:], 0.0)
    chunked_load(nc, eexpf, emb, eng=nc.gpsimd)

    ident = const.tile([128, 128], BF16)
    make_identity(nc, ident)

    biasg = const.tile([128, NCHUNK, 2 * DOUT], BF16)
    biasu = const.tile([128, NCHUNK, DOUT], BF16)
    nc.vector.memset(biasg[64:, NCHUNK - 1], 0.0)
    nc.vector.memset(biasu[64:, NCHUNK - 1], 0.0)
    rinv = const.tile([128, NCHUNK], F32)
    dsum_all = const.tile([128, NCHUNK], F32)

    # WPg [128, 3, 1024] bf16: rows 0:96 = c, cols (d,o) d-major.
    WPg = const.tile([128, K, EMB * 64], BF16)
    nc.vector.memset(WPg[:], 0.0)
    WPu = const.tile([128, K, EMB * 32], BF16)
    nc.vector.memset(WPu[:], 0.0)
    WWg = const.tile([128, EMB * 64], BF16)
    nc.vector.memset(WWg[:], 0.0)
    WWu = const.tile([128, EMB * 32], BF16)
    nc.vector.memset(WWu[:], 0.0)

    Tb = const.tile([128, 2, WLEN], F32)
    for w, name in ((0, "gT"), (1, "uT")):
        src = io[name][:]
        nc.sync.dma_start(
            Tb[:, w, :],
            bass.AP(tensor=src.tensor, offset=src.offset, ap=[[0, 128]] + list(src.ap)),
        )

    X1 = big.tile([128, NCHUNK, B_LOC, 128], BF16, tag="slot1")
    X2 = big.tile([128, NCHUNK, B_LOC, 128], BF16, tag="slot2")
    X3 = big.tile([128, NCHUNK, B_LOC, 128], BF16, tag="slot3")
    nc.vector.memset(X1[:], 0.0)
    nc.gpsimd.memset(X2[:], 0.0)
    nc.gpsimd.memset(X3[:], 0.0)
    XtT = big.tile([128, B_LOC * NPAD], BF16, tag="XtT")
    r_gate = big.tile([128, NCHUNK, B_LOC, DOUT], BF16, tag="rgate")
    A = big.tile([128, NCHUNK, N], BF16, tag="A")

    FLAT = N * CW // 128  # 750

    with tc.tile_pool(name="stage", bufs=3) as stage, \
         tc.tile_pool(name="stage3", bufs=2) as stage3, \
         tc.tile_pool(name="xtacc", bufs=1) as xtacc:
        # ---- weight pools load/pack
        for k in range(K):
            wk = stage.tile([128, EMB, 64], F32, tag="stg")
            nc.sync.dma_start(wk[0:CIN], io["gw_pool"][:, k].rearrange("d c o -> c d o"))
            nc.vector.tensor_copy(
                WPg[0:CIN, k].rearrange("p (d o) -> p d o", d=EMB), wk[0:CIN]
            )
            wku = stage.tile([128, EMB, 32], F32, tag="stg")
            nc.sync.dma_start(wku[0:32], io["uw_pool"][:, k, 0:32, :].rearrange("d c o -> c d o"))
            nc.sync.dma_start(wku[64:128], io["uw_pool"][:, k, 32:96, :].rearrange("d c o -> c d o"))
            nc.vector.tensor_copy(
                WPu[0:32, k].rearrange("p (d o) -> p d o", d=EMB), wku[0:32]
            )
            nc.vector.tensor_copy(
                WPu[64:128, k].rearrange("p (d o) -> p d o", d=EMB), wku[64:128]
            )
        wg = stage.tile([128, EMB, 64], F32, tag="stg")
        nc.sync.dma_start(wg[0:CW], io["gw_win"].rearrange("d i o -> i d o"))
        nc.vector.tensor_copy(WWg[0:CW].rearrange("p (d o) -> p d o", d=EMB), wg[0:CW])
        wu = stage.tile([128, EMB, 32], F32, tag="stg")
        # rows 64:112 (matches packed XtT where xt_u.T sits at partitions 64:112)
        nc.sync.dma_start(wu[64 : 64 + CW], io["uw_win"].rearrange("d i o -> i d o"))
        nc.vector.tensor_copy(
            WWu[64 : 64 + CW].rearrange("p (d o) -> p d o", d=EMB), wu[64 : 64 + CW]
        )

        # ---- biases + A
        with tc.tile_pool(name="prep", bufs=1) as prep:
            embT_raw = prep.tile([EMB, N], F32)
            nc.sync.dma_start(embT_raw[:], emb.rearrange("n d -> d n"))
            embT = prep.tile([EMB, N], F32R)
            nc.vector.tensor_copy(embT[:], embT_raw[:])
            gbp_raw = prep.tile([EMB, 2 * DOUT], F32)
            nc.sync.dma_start(gbp_raw[:], io["gb_pool"][:])
            gbp_s = prep.tile([EMB, 2 * DOUT], F32R)
            nc.vector.tensor_copy(gbp_s[:], gbp_raw[:])
            ubp_raw = prep.tile([EMB, DOUT], F32)
            nc.sync.dma_start(ubp_raw[:], io["ub_pool"][:])
            ubp_s = prep.tile([EMB, DOUT], F32R)
            nc.vector.tensor_copy(ubp_s[:], ubp_raw[:])
            with tc.tile_pool(name="psum_pre", bufs=2, space="PSUM") as psum_pre:
                for nch in range(NCHUNK):
                    l = nlen(nch)
                    nsl = slice(nch * 128, nch * 128 + l)
                    pg = psum_pre.tile([128, N], F32, tag="pg")
                    for mj in range(4):
                        m0 = mj * 512
                        mw = min(512, N - m0)
                        nc.tensor.matmul(
                            pg[:l, m0 : m0 + mw], embT[:, nsl],
                            embT[:, m0 : m0 + mw], start=True, stop=True,
                        )
                    nc.scalar.activation(A[:l, nch, :], pg[:l, :], AF.Exp)
                    nc.vector.tensor_scalar(
                        out=A[:l, nch, :], in0=A[:l, nch, :],
                        scalar1=1.0, scalar2=0.0, op0=OP.max, op1=OP.add,
                        accum_out=dsum_all[:l, nch : nch + 1],
                    )
                    nc.vector.reciprocal(rinv[:l, nch : nch + 1], dsum_all[:l, nch : nch + 1])
            with tc.tile_pool(name="psum_b", bufs=2, space="PSUM") as psum_b:
                for nch in range(NCHUNK):
                    l = nlen(nch)
                    nsl = slice(nch * 128, nch * 128 + l)
                    pb = psum_b.tile([128, 3 * DOUT], F32, tag="pbias")
                    nc.tensor.matmul(
                        pb[:l, 0 : 2 * DOUT], embT[:, nsl],
                        gbp_s[:], start=True, stop=True,
                    )
                    nc.tensor.matmul(
                        pb[:l, 2 * DOUT :], embT[:, nsl],
                        ubp_s[:], start=True, stop=True,
                    )
                    nc.scalar.copy(biasg[:l, nch, :], pb[:l, 0 : 2 * DOUT])
                    nc.scalar.copy(biasu[:l, nch, :], pb[:l, 2 * DOUT :])

        # ---- x/state -> X1 [x | state] bf16
        for b in range(B_LOC):
            xs = stage.tile([128, NCHUNK, DIN], F32, tag="stg")
            nc.vector.memset(xs[64:, NCHUNK - 1], 0.0)
            chunked_load(nc, xs, x[b])
            nc.vector.tensor_copy(X1[:, :, b, 0:DIN], xs[:])
            ss = stage.tile([128, NCHUNK, DOUT], F32, tag="stg")
            nc.vector.memset(ss[64:, NCHUNK - 1], 0.0)
            chunked_load(nc, ss, state[b])
            nc.scalar.copy(X1[:, :, b, DIN:CIN], ss[:])

        # ---- window t-contraction (flat layout), then DRAM-bounce into XtT
        zeros128 = const.tile([128, 128], BF16)
        nc.vector.memset(zeros128[:], 0.0)
        dzero = dram.tile([NPAD, 128], BF16, tag="dzero")
        nc.sync.dma_start(
            dzero.rearrange("(c p) o -> p c o", p=128),
            bass.AP(tensor=zeros128.tensor, offset=zeros128.offset,
                    ap=[[1, 128], [0, NCHUNK], [1, 128]]),
        )
        xt_g = xtacc.tile([128, B_LOC, FLAT], F32)
        xt_u = xtacc.tile([128, B_LOC, FLAT], BF16)
        # b-outer so each batch's XtT panel packs as soon as its 12 t-loads
        # land, overlapping the remaining loads and the gate diffusion
        for b in range(B_LOC):
            for t in range(WLEN):
                st = stage3.tile([128, FLAT], F32, tag="xw")
                nc.sync.dma_start(
                    st[:],
                    x_full[b, t].rearrange("n i -> (n i)").rearrange("(p f) -> p f", p=128),
                )
                for w, acc in ((0, xt_g), (1, xt_u)):
                    if t == 0:
                        nc.vector.tensor_scalar(
                            out=acc[:, b, :], in0=st[:],
                            scalar1=Tb[:, w, 0:1], scalar2=None, op0=OP.mult,
                        )
                    else:
                        nc.vector.scalar_tensor_tensor(
                            out=acc[:, b, :], in0=st[:],
                            scalar=Tb[:, w, t : t + 1],
                            in1=acc[:, b, :], op0=OP.mult, op1=OP.add,
                        )
            xgb16 = stage.tile([128, 2, FLAT], BF16, tag="stg")
            nc.gpsimd.tensor_copy(xgb16[:, 0, :], xt_g[:, b, :])
            nc.gpsimd.tensor_copy(xgb16[:, 1, :], xt_u[:, b, :])
            dflat = dram.tile([2, 128, FLAT], BF16, tag="dflat")
            nc.sync.dma_start(dflat.rearrange("w p f -> p w f"), xgb16[:])
            dpan = dram.tile([NPAD, 128], BF16, tag="pan")
            dfv = dflat.rearrange("w p f -> w (p f)").rearrange("w (n i) -> w n i", n=N)
            nc.sync.dma_start(dpan[0:N, 0:CW], dfv[0])
            nc.sync.dma_start(dpan[0:N, 64 : 64 + CW], dfv[1])
            nc.sync.dma_start(dpan[0:N, CW:64], dzero[0:N, 0:16])
            nc.sync.dma_start(dpan[0:N, 112:128], dzero[0:N, 0:16])
            nc.sync.dma_start(dpan[N:NPAD, :], dzero[N:NPAD, :])
            nc.sync.dma_start(XtT[:, b * NPAD : (b + 1) * NPAD], dpan[:], transpose=True)

        # ---- gate diffusion (A resident): nch-outer, psum accumulate over mi
        with tc.tile_pool(name="psum_d1", bufs=3, space="PSUM") as psum_d1:
            for SRC, DST in ((X1, X2), (X2, X3)):
                for nch in range(NCHUNK):
                    l = nlen(nch)
                    ph = psum_d1.tile([128, B_LOC, CIN], F32, tag="pdiff")
                    for mi in range(NCHUNK):
                        ml = nlen(mi)
                        nc.tensor.matmul(
                            ph[:l], A[:ml, mi, nch * 128 : nch * 128 + l],
                            SRC[:ml, mi, :, 0:CIN],
                            start=(mi == 0), stop=(mi == NCHUNK - 1),
                        )
                    nc.scalar.activation(
                        DST[:l, nch, :, 0:CIN], ph[:l],
                        AF.Copy, scale=rinv[:l, nch : nch + 1],
                    )
    # Apool/stage/xtacc closed: A + staging SBUF freed for the y phases.

    # ================= shared y-phase pools =================
    acc_pool = ctx.enter_context(tc.tile_pool(name="accp", bufs=2))
    tmp_pool = ctx.enter_context(tc.tile_pool(name="tmpp", bufs=2))
    tail_pool = ctx.enter_context(tc.tile_pool(name="tailp", bufs=2))
    ysu_pool = ctx.enter_context(tc.tile_pool(name="ysu", bufs=1))
    xtb_pool = ctx.enter_context(tc.tile_pool(name="xtb", bufs=3))

    def dred4(ysh, nblk, owid, nch, bias):
        """Batched d-reduction over all 4 b: returns acc [128, B_LOC, nblk*owid].

        ysh: two half-tiles [128, B_LOC, nblk*owid*8] bf16, ysh[h] holding
        y d-slices 8h..8h+7, per-b cols [blk0 8d x owid | blk1 8d x owid].
        bias: [128, nblk*owid] (seeded per-b on the DVE chain's d=0).
        """
        # Decomposed as 16 DVE tensor_scalar muls (4x-mode capable, unlike
        # scalar_tensor_tensor which has no DVE perf modes) + an add chain
        # split between DVE (accA: t0..t11 + per-b bias) and GPSIMD
        # (accB: t12..t15, then the accA+accB merge).
        W = nblk * owid
        N_POOL = 4  # trailing d-terms accumulated on GPSIMD
        accAf = acc_pool.tile([128, B_LOC, 128], BF16, tag="accA")
        accBf = acc_pool.tile([128, B_LOC, 128], BF16, tag="accB")
        accA, accB = accAf[:, :, 0:W], accBf[:, :, 0:W]
        dper = EMB // len(ysh)
        ys = [t.rearrange("p b (blk d o) -> p b blk d o", blk=nblk, d=dper)
              for t in ysh]
        bias3 = bias.rearrange("p (blk o) -> p blk o", blk=nblk)
        accA4 = accA.rearrange("p b (blk o) -> p b blk o", blk=nblk)
        t0f = tmp_pool.tile([128, B_LOC, 128], BF16, tag="t0")
        t1f = tmp_pool.tile([128, B_LOC, 128], BF16, tag="t1")
        t2f = tmp_pool.tile([128, B_LOC, 128], BF16, tag="t2")
        tt = [t0f[:, :, 0:W], t1f[:, :, 0:W], t2f[:, :, 0:W]]
        nd = EMB - N_POOL
        for d in range(EMB):
            src = ys[d // dper][:, :, :, d % dper, :]
            if d == 0:
                nc.vector.tensor_scalar(
                    out=accA[:], in0=src,
                    scalar1=eexpf[:, nch, 0:1], scalar2=None, op0=OP.mult,
                )
            elif d == nd:
                nc.vector.tensor_scalar(
                    out=accB[:], in0=src,
                    scalar1=eexpf[:, nch, d : d + 1], scalar2=None, op0=OP.mult,
                )
            else:
                t = tt[d % 3]
                nc.vector.tensor_scalar(
                    out=t[:], in0=src,
                    scalar1=eexpf[:, nch, d : d + 1], scalar2=None, op0=OP.mult,
                )
                eng = nc.vector if d < nd else nc.gpsimd
                eng.tensor_tensor(
                    out=accA[:] if d < nd else accB[:],
                    in0=accA[:] if d < nd else accB[:],
                    in1=t[:], op=OP.add,
                )
        for b in range(B_LOC):
            nc.vector.tensor_tensor(
                out=accA4[:, b], in0=accA4[:, b], in1=bias3[:], op=OP.add,
            )
        nc.gpsimd.tensor_tensor(out=accA[:], in0=accA[:], in1=accB[:], op=OP.add)
        return accA

    # ================= gate y-GEMM (nch-outer, b-batched tail) =================
    with tc.tile_pool(name="ysg", bufs=2) as ysg_pool, \
         tc.tile_pool(name="psum_yg", bufs=3, space="PSUM") as psum_yg, \
         tc.tile_pool(name="psum_tg", bufs=2, space="PSUM") as psum_tg:
        def transpose_batch(srcs, nch, tag):
            # PE transposes: XkT for all 4 b; two 1-bank psum halves so the
            # y-matmul pool can take 3 bufs (6 banks)
            xtb = xtb_pool.tile([128, 3, B_LOC, 128], BF16, tag="xtb")
            xv = xtb.rearrange("p k b n -> p (k b) n")
            for h in range(2):
                pt = psum_tg.tile([128, 6 * 128], BF16, tag=f"pt{tag}")
                for j in range(6):
                    ki, b = divmod(h * 6 + j, B_LOC)
                    nc.tensor.transpose(
                        pt[:, j * 128 : (j + 1) * 128],
                        srcs[ki][:, nch, b, :], ident[:]
                    )
                nc.vector.tensor_copy(xv[:, h * 6 : (h + 1) * 6, :], pt[:])
            return xtb

        def gate_tail(nch, acc):
            acc4 = acc.rearrange("p b (blk o) -> p b blk o", blk=2)
            ztile = acc_pool.tile([128, B_LOC, DOUT], BF16, tag="ztile")
            nc.scalar.activation(ztile[:], acc4[:, :, 0, :], AF.Sigmoid)
            nc.scalar.activation(r_gate[:, nch], acc4[:, :, 1, :], AF.Sigmoid)
            # zs = z*state: stage the state slice first so the X1 write
            # never overlaps its own read range.
            zsrc = acc_pool.tile([128, B_LOC, DOUT], BF16, tag="zsrc")
            nc.gpsimd.tensor_copy(zsrc[:], X1[:, nch, :, DIN:CIN])
            nc.gpsimd.tensor_mul(X1[:, nch, :, 64:128], ztile[:], zsrc[:])

        PF = 2  # transpose prefetch distance (chunks)
        pend = []
        xtb_q = [transpose_batch((X1, X2, X3), j, "g") for j in range(PF)]
        for nch in range(NCHUNK):
            l = nlen(nch)
            if nch + PF < NCHUNK:
                xtb_q.append(transpose_batch((X1, X2, X3), nch + PF, "g"))
            xtb = xtb_q.pop(0)
            # --- y matmuls: per (b, half) psum [128, 1024]
            ysh0 = ysg_pool.tile([128, B_LOC, 1024], BF16, tag="ysg")
            ysh1 = ysg_pool.tile([128, B_LOC, 1024], BF16, tag="ysg")
            ysh = (ysh0, ysh1)
            for half in range(2):
                for b in range(B_LOC):
                    py = psum_yg.tile([128, 1024], F32, tag="pyg")
                    hs = slice(half * 512, half * 512 + 512)
                    for k in range(K):
                        nc.tensor.matmul(
                            py[:, 0:512], xtb[0:CIN, k, b, :],
                            WPg[0:CIN, k, hs],
                            start=(k == 0), stop=(k == K - 1),
                        )
                    nc.tensor.matmul(
                        py[:, 512:1024],
                        XtT[:, b * NPAD + nch * 128 : b * NPAD + nch * 128 + 128],
                        WWg[:, hs], start=True, stop=True,
                    )
                    nc.scalar.copy(ysh[half][:, b, :], py[:, 0:1024])
            # --- batched d-reduction; tail deferred one chunk so the next
            # chunk's PSUM evicts aren't queued behind sigmoid on ACT
            acc = dred4(ysh, 2, DOUT, nch, biasg[:, nch])
            pend.append((nch, acc))
            if len(pend) > 1:
                gate_tail(*pend.pop(0))

        while pend:
            gate_tail(*pend.pop(0))

    CAND = X1  # panels now hold [x | state(stale) | z*state]

    # ================= update diffusion =================
    C2, C3 = X2, X3
    with tc.tile_pool(name="psum_d2", bufs=3, space="PSUM") as psum_d2:
        for SRC, DST in ((CAND, C2), (C2, C3)):
            for nch in range(NCHUNK):
                l = nlen(nch)
                ph = psum_d2.tile([128, B_LOC, DOUT], F32, tag="pdiff2")
                for mi in range(NCHUNK):
                    ml = nlen(mi)
                    nc.tensor.matmul(
                        ph[:l], A[:ml, mi, nch * 128 : nch * 128 + l],
                        SRC[:ml, mi, :, 64:128],
                        start=(mi == 0), stop=(mi == NCHUNK - 1),
                    )
                nc.scalar.activation(
                    DST[:l, nch, :, 64:128], ph[:l],
                    AF.Copy, scale=rinv[:l, nch : nch + 1],
                )

    # ================= update y-GEMM + output =================
    with tc.tile_pool(name="psum_yu", bufs=3, space="PSUM") as psum_yu, \
         tc.tile_pool(name="psum_tu", bufs=2, space="PSUM") as psum_tu:
        def transpose_batch_u(nch):
            srcs = (CAND, C2, C3)
            xtb = xtb_pool.tile([128, 3, B_LOC, 128], BF16, tag="xtb")
            xv = xtb.rearrange("p k b n -> p (k b) n")
            for h in range(2):
                pt = psum_tu.tile([128, 6 * 128], BF16, tag="ptu")
                for j in range(6):
                    ki, b = divmod(h * 6 + j, B_LOC)
                    nc.tensor.transpose(
                        pt[:, j * 128 : (j + 1) * 128],
                        srcs[ki][:, nch, b, :], ident[:]
                    )
                nc.vector.tensor_copy(xv[:, h * 6 : (h + 1) * 6, :], pt[:])
            return xtb

        def upd_tail(nch, accu):
            l = nlen(nch)
            hc = tail_pool.tile([128, B_LOC, DOUT], F32, tag="hc")
            nc.scalar.activation(hc[:], accu[:], AF.Tanh)
            stf = tail_pool.tile([128, B_LOC, DOUT], F32, tag="stf")
            if l < 128:
                nc.vector.memset(stf[64:], 0.0)
            for b in range(B_LOC):
                nc.sync.dma_start(stf[:l, b], state[b, nch * 128 : nch * 128 + l, :])
            tmp = tail_pool.tile([128, B_LOC, DOUT], F32, tag="tmp")
            nc.gpsimd.tensor_sub(tmp[:], stf[:], hc[:])
            nc.gpsimd.tensor_mul(tmp[:], tmp[:], r_gate[:, nch])
            nc.gpsimd.tensor_add(tmp[:], tmp[:], hc[:])
            for b in range(B_LOC):
                nc.sync.dma_start(
                    out[b, nch * 128 : nch * 128 + l, :], tmp[:l, b, :]
                )

        PF = 2
        pend = []
        xtb_q = [transpose_batch_u(j) for j in range(PF)]
        for nch in range(NCHUNK):
            l = nlen(nch)
            if nch + PF < NCHUNK:
                xtb_q.append(transpose_batch_u(nch + PF))
            xtb = xtb_q.pop(0)
            ysu = ysu_pool.tile([128, B_LOC, 1024], BF16, tag="ysu")
            for b in range(B_LOC):
                pu = psum_yu.tile([128, 1024], F32, tag="pyu")
                for k in range(K):
                    nc.tensor.matmul(
                        pu[:, 0:512], xtb[:, k, b, :], WPu[:, k, :],
                        start=(k == 0), stop=(k == K - 1),
                    )
                nc.tensor.matmul(
                    pu[:, 512:1024],
                    XtT[:, b * NPAD + nch * 128 : b * NPAD + nch * 128 + 128],
                    WWu[:], start=True, stop=True,
                )
                nc.scalar.copy(ysu[:, b, :], pu[:, 0:1024])
            accu = dred4([ysu], 2, 32, nch, biasu[:, nch])
            pend.append((nch, accu))
            if len(pend) > 1:
                upd_tail(*pend.pop(0))
        while pend:
            upd_tail(*pend.pop(0))


def make_io(nc):
    io = {}
    io["x"] = nc.dram_tensor("x", [B_LOC, N, DIN], F32, kind="ExternalInput")
    io["state"] = nc.dram_tensor("state", [B_LOC, N, DOUT], F32, kind="ExternalInput")
    io["x_full"] = nc.dram_tensor("x_full", [B_LOC, WLEN, N, CW], F32, kind="ExternalInput")
    io["node_embeddings"] = nc.dram_tensor("node_embeddings", [N, EMB], F32, kind="ExternalInput")
    io["gw_pool"] = nc.dram_tensor("gw_pool", [EMB, K, CIN, 64], F32, kind="ExternalInput")
    io["gw_win"] = nc.dram_tensor("gw_win", [EMB, CW, 64], F32, kind="ExternalInput")
    io["gb_pool"] = nc.dram_tensor("gb_pool", [EMB, 2 * DOUT], F32, kind="ExternalInput")
    io["gT"] = nc.dram_tensor("gT", [WLEN], F32, kind="ExternalInput")
    io["uw_pool"] = nc.dram_tensor("uw_pool", [EMB, K, CIN, 32], F32, kind="ExternalInput")
    io["uw_win"] = nc.dram_tensor("uw_win", [EMB, CW, 32], F32, kind="ExternalInput")
    io["ub_pool"] = nc.dram_tensor("ub_pool", [EMB, DOUT], F32, kind="ExternalInput")
    io["uT"] = nc.dram_tensor("uT", [WLEN], F32, kind="ExternalInput")
    io["out"] = nc.dram_tensor("out", [B_LOC, N, DOUT], F32, kind="ExternalOutput")
    return io


def build_module(debug=False):
    from concourse import bacc

    nc = bacc.Bacc("TRN2", target_bir_lowering=False, debug=debug)
    io = make_io(nc)
    with tile.TileContext(nc) as tc:
        build(tc, io)
    nc.finalize()
    return nc


# ======================= harness wrapper =======================
import numpy as _np

N_CORES = 8
_CACHE = {}


def _get_module():
    if "nc" not in _CACHE:
        _CACHE["nc"] = build_module()
    return _CACHE["nc"]


def make_in_maps(inputs):
    xb = _np.ascontiguousarray(inputs["x"], dtype=_np.float32)
    sb = _np.ascontiguousarray(inputs["state"], dtype=_np.float32)
    xf = _np.ascontiguousarray(inputs["x_full"], dtype=_np.float32)
    rep = {
        k: _np.ascontiguousarray(inputs[k], dtype=_np.float32)
        for k in ("node_embeddings", "gw_pool", "gw_win", "gb_pool", "gT",
                  "uw_pool", "uw_win", "ub_pool", "uT")
    }
    in_maps = []
    for i in range(N_CORES):
        m = dict(rep)
        m["x"] = xb[i * B_LOC : (i + 1) * B_LOC]
        m["state"] = sb[i * B_LOC : (i + 1) * B_LOC]
        m["x_full"] = xf[i * B_LOC : (i + 1) * B_LOC]
        in_maps.append(m)
    return in_maps


def kernel(**inputs):
    """Full-input entry point: shards over batch across 8 NeuronCores."""
    nc = _get_module()
    from concourse.bass_utils import run_bass_kernel_spmd

    in_maps = make_in_maps(inputs)
    res = run_bass_kernel_spmd(nc, in_maps, core_ids=list(range(N_CORES)))
    return _np.concatenate([res.results[i]["out"] for i in range(N_CORES)], axis=0)



# revision 12
# speedup vs baseline: 1.0111x; 1.0111x over previous
import sys
for _p in ('/opt/trn_rl_repo',):
    if _p not in sys.path:
        sys.path.insert(0, _p)

"""NLSGCRN cell Bass/Tile kernel for TRN2, batch-sharded SPMD over 8 cores.

Per-core shapes (b_loc = 4 batches):
  x [4,2000,32], state [4,2000,64], x_full [4,12,2000,48], emb [2000,16],
  pools gw/uw/gb/ub/gT/uT, out h [4,2000,64].

v2 structure (vs v1 baseline):
- y-phases loop nch-outer / b-inner: the per-node d-reduction batches all
  4 batches into one op (the e[n,d] scalar is per-partition and shared
  across b), split DVE (d < D_DVE, seeded with bias) / GPSIMD (rest).
- XkT stationaries via PE transposes (identity matmul) into bf16 PSUM,
  evicted by DVE, replacing v1's DRAM-bounce DMA transposes.
- A is spilled to DRAM at generation and its SBUF tile is freed after the
  gate hops; update diffusion streams A chunks back mi-outer with all 16
  output-chunk accumulators resident in PSUM (8 banks exactly).
- PSUM bank budget in y phases: y tiles [128,1024] f32 (2 banks) x2 bufs
  + transpose batch [128,1536] bf16 (2 banks) x2 bufs.
"""

from contextlib import ExitStack

import concourse.bass as bass
import concourse.tile as tile
from concourse import mybir
from concourse._compat import with_exitstack
from concourse.masks import make_identity

F32 = mybir.dt.float32
F32R = mybir.dt.float32r
BF16 = mybir.dt.bfloat16
AF = mybir.ActivationFunctionType
OP = mybir.AluOpType

B_LOC = 4
N = 2000
NCHUNK = 16           # ceil(2000/128)
NFULL = (NCHUNK - 1) * 128   # 1920
NPAD = NCHUNK * 128   # 2048
DIN, DOUT = 32, 64
CIN = 96
CW = 48
WLEN = 12
EMB = 16
K = 3

D_DVE = 8   # d-reduction: first D_DVE iterations on DVE, rest on GPSIMD


def nlen(nch):
    return 128 if nch < NCHUNK - 1 else N - NFULL  # last = 80


def chunked_load(nc, dst, src, eng=None):
    """dst [128, NCHUNK, ...inner] <- src [2000, ...inner] splitting rows."""
    eng = eng or nc.sync
    inner = src.shape[1:]
    eng.dma_start(
        dst[:, 0 : NCHUNK - 1],
        src[0:NFULL].rearrange(
            "(c p) " + " ".join(f"i{j}" for j in range(len(inner)))
            + " -> p c " + " ".join(f"i{j}" for j in range(len(inner))),
            p=128,
        ),
    )
    eng.dma_start(dst[0 : N - NFULL, NCHUNK - 1], src[NFULL:N])


@with_exitstack
def build(ctx: ExitStack, tc: tile.TileContext, io: dict):
    nc = tc.nc

    io = {k: (v[:] if not isinstance(v, bass.AP) else v) for k, v in io.items()}
    x, state, x_full = io["x"], io["state"], io["x_full"]
    emb = io["node_embeddings"]
    out = io["out"]

    const = ctx.enter_context(tc.tile_pool(name="const", bufs=1))
    big = ctx.enter_context(tc.tile_pool(name="big", bufs=1))
    dram = ctx.enter_context(tc.tile_pool(name="dram", bufs=6, space="DRAM"))

    # ================= constants / weights =================
    eexpf = const.tile([128, NCHUNK, EMB], F32)
    nc.vector.memset(eexpf[:], 0.0)
    chunked_load(nc, eexpf, emb, eng=nc.gpsimd)

    ident = const.tile([128, 128], BF16)
    make_identity(nc, ident)

    biasg = const.tile([128, NCHUNK, 2 * DOUT], BF16)
    biasu = const.tile([128, NCHUNK, DOUT], BF16)
    nc.vector.memset(biasg[64:, NCHUNK - 1], 0.0)
    nc.vector.memset(biasu[64:, NCHUNK - 1], 0.0)
    rinv = const.tile([128, NCHUNK], F32)
    dsum_all = const.tile([128, NCHUNK], F32)

    # WPg [128, 3, 1024] bf16: rows 0:96 = c, cols (d,o) d-major.
    WPg = const.tile([128, K, EMB * 64], BF16)
    nc.vector.memset(WPg[:], 0.0)
    WPu = const.tile([128, K, EMB * 32], BF16)
    nc.vector.memset(WPu[:], 0.0)
    WWg = const.tile([128, EMB * 64], BF16)
    nc.vector.memset(WWg[:], 0.0)
    WWu = const.tile([128, EMB * 32], BF16)
    nc.vector.memset(WWu[:], 0.0)

    Tb = const.tile([128, 2, WLEN], F32)
    for w, name in ((0, "gT"), (1, "uT")):
        src = io[name][:]
        nc.sync.dma_start(
            Tb[:, w, :],
            bass.AP(tensor=src.tensor, offset=src.offset, ap=[[0, 128]] + list(src.ap)),
        )

    X1 = big.tile([128, NCHUNK, B_LOC, 128], BF16, tag="slot1")
    X2 = big.tile([128, NCHUNK, B_LOC, 128], BF16, tag="slot2")
    X3 = big.tile([128, NCHUNK, B_LOC, 128], BF16, tag="slot3")
    nc.vector.memset(X1[:], 0.0)
    nc.gpsimd.memset(X2[:], 0.0)
    nc.gpsimd.memset(X3[:], 0.0)
    XtT = big.tile([128, B_LOC * NPAD], BF16, tag="XtT")
    r_gate = big.tile([128, NCHUNK, B_LOC, DOUT], BF16, tag="rgate")
    A = big.tile([128, NCHUNK, N], BF16, tag="A")

    FLAT = N * CW // 128  # 750

    with tc.tile_pool(name="stage", bufs=3) as stage, \
         tc.tile_pool(name="stage3", bufs=3) as stage3, \
         tc.tile_pool(name="xtacc", bufs=1) as xtacc:
      with tc.tile_pool(name="prep", bufs=1) as prep, \
           tc.tile_pool(name="psum_pre", bufs=1, space="PSUM") as psum_pre, \
           tc.tile_pool(name="psum_b", bufs=2, space="PSUM") as psum_b:
        # ---- small loads all go through the gpsimd SWDGE queue (which can
        # also cast f32->bf16 in flight); nc.sync's HWDGE queue is reserved
        # for the x_full stream + XtT bounce.
        embT_raw = prep.tile([EMB, N], F32)
        nc.sync.dma_start(embT_raw[:], emb.rearrange("n d -> d n"))
        embT = prep.tile([EMB, N], F32R)
        nc.vector.tensor_copy(embT[:], embT_raw[:])
        gbp_raw = prep.tile([EMB, 2 * DOUT], F32)
        nc.gpsimd.dma_start(gbp_raw[:], io["gb_pool"][:])
        gbp_s = prep.tile([EMB, 2 * DOUT], F32R)
        nc.vector.tensor_copy(gbp_s[:], gbp_raw[:])
        ubp_raw = prep.tile([EMB, DOUT], F32)
        nc.gpsimd.dma_start(ubp_raw[:], io["ub_pool"][:])
        ubp_s = prep.tile([EMB, DOUT], F32R)
        nc.vector.tensor_copy(ubp_s[:], ubp_raw[:])

        # ---- x/state -> X1 [x | state] bf16 via casting SWDGE DMAs
        for b in range(B_LOC):
            chunked_load(nc, X1[:, :, b, 0:DIN], x[b], eng=nc.gpsimd)
            chunked_load(nc, X1[:, :, b, DIN:CIN], state[b], eng=nc.gpsimd)

        # ---- weight pools: casting SWDGE DMAs straight into packed tiles
        for k in range(K):
            nc.gpsimd.dma_start(
                WPg[0:CIN, k].rearrange("p (d o) -> p d o", d=EMB),
                io["gw_pool"][:, k].rearrange("d c o -> c d o"),
            )
            nc.gpsimd.dma_start(
                WPu[0:32, k].rearrange("p (d o) -> p d o", d=EMB),
                io["uw_pool"][:, k, 0:32, :].rearrange("d c o -> c d o"),
            )
            nc.gpsimd.dma_start(
                WPu[64:128, k].rearrange("p (d o) -> p d o", d=EMB),
                io["uw_pool"][:, k, 32:96, :].rearrange("d c o -> c d o"),
            )
        nc.gpsimd.dma_start(
            WWg[0:CW].rearrange("p (d o) -> p d o", d=EMB),
            io["gw_win"].rearrange("d i o -> i d o"),
        )
        # rows 64:112 (matches packed XtT where xt_u.T sits at partitions 64:112)
        nc.gpsimd.dma_start(
            WWu[64 : 64 + CW].rearrange("p (d o) -> p d o", d=EMB),
            io["uw_win"].rearrange("d i o -> i d o"),
        )

        def agen_chunk(nch):
            l = nlen(nch)
            nsl = slice(nch * 128, nch * 128 + l)
            pg = psum_pre.tile([128, N], F32, tag="pg")
            for mj in range(4):
                m0 = mj * 512
                mw = min(512, N - m0)
                nc.tensor.matmul(
                    pg[:l, m0 : m0 + mw], embT[:, nsl],
                    embT[:, m0 : m0 + mw], start=True, stop=True,
                )
            nc.scalar.activation(A[:l, nch, :], pg[:l, :], AF.Exp)
            nc.vector.tensor_scalar(
                out=A[:l, nch, :], in0=A[:l, nch, :],
                scalar1=1.0, scalar2=0.0, op0=OP.max, op1=OP.add,
                accum_out=dsum_all[:l, nch : nch + 1],
            )
            nc.vector.reciprocal(rinv[:l, nch : nch + 1], dsum_all[:l, nch : nch + 1])

        def bias_chunk(nch):
            l = nlen(nch)
            nsl = slice(nch * 128, nch * 128 + l)
            pb = psum_b.tile([128, 3 * DOUT], F32, tag="pbias")
            nc.tensor.matmul(
                pb[:l, 0 : 2 * DOUT], embT[:, nsl], gbp_s[:], start=True, stop=True,
            )
            nc.tensor.matmul(
                pb[:l, 2 * DOUT :], embT[:, nsl], ubp_s[:], start=True, stop=True,
            )
            nc.scalar.copy(biasg[:l, nch, :], pb[:l, 0 : 2 * DOUT])
            nc.scalar.copy(biasu[:l, nch, :], pb[:l, 2 * DOUT :])

        # ---- window t-contraction (flat layout) + DRAM-bounce into XtT,
        # with A-gen / bias chunks interleaved so their DVE/ACT/PE work
        # overlaps the x_full DMA stream.
        zeros128 = const.tile([128, 128], BF16)
        nc.vector.memset(zeros128[:], 0.0)
        dzero = dram.tile([NPAD, 128], BF16, tag="dzero")
        nc.sync.dma_start(
            dzero.rearrange("(c p) o -> p c o", p=128),
            bass.AP(tensor=zeros128.tensor, offset=zeros128.offset,
                    ap=[[1, 128], [0, NCHUNK], [1, 128]]),
        )
        xt_g = xtacc.tile([128, B_LOC, FLAT], F32)
        xt_u = xtacc.tile([128, B_LOC, FLAT], BF16)
        # b-outer so each batch's XtT panel packs as soon as its 12 t-loads
        # land, overlapping the remaining loads and the gate diffusion
        for b in range(B_LOC):
            for t in range(WLEN):
                st = stage3.tile([128, FLAT], F32, tag="xw")
                nc.sync.dma_start(
                    st[:],
                    x_full[b, t].rearrange("n i -> (n i)").rearrange("(p f) -> p f", p=128),
                )
                for w, acc in ((0, xt_g), (1, xt_u)):
                    if t == 0:
                        nc.vector.tensor_scalar(
                            out=acc[:, b, :], in0=st[:],
                            scalar1=Tb[:, w, 0:1], scalar2=None, op0=OP.mult,
                        )
                    else:
                        nc.vector.scalar_tensor_tensor(
                            out=acc[:, b, :], in0=st[:],
                            scalar=Tb[:, w, t : t + 1],
                            in1=acc[:, b, :], op0=OP.mult, op1=OP.add,
                        )
                idx = b * WLEN + t
                if idx < NCHUNK:
                    agen_chunk(idx)
                elif idx < 2 * NCHUNK:
                    bias_chunk(idx - NCHUNK)
            xgb16 = stage.tile([128, 2, FLAT], BF16, tag="stg")
            nc.gpsimd.tensor_copy(xgb16[:, 0, :], xt_g[:, b, :])
            nc.gpsimd.tensor_copy(xgb16[:, 1, :], xt_u[:, b, :])
            dflat = dram.tile([2, 128, FLAT], BF16, tag="dflat")
            nc.sync.dma_start(dflat.rearrange("w p f -> p w f"), xgb16[:])
            dpan = dram.tile([NPAD, 128], BF16, tag="pan")
            dfv = dflat.rearrange("w p f -> w (p f)").rearrange("w (n i) -> w n i", n=N)
            nc.sync.dma_start(dpan[0:N, 0:CW], dfv[0])
            nc.sync.dma_start(dpan[0:N, 64 : 64 + CW], dfv[1])
            nc.sync.dma_start(dpan[0:N, CW:64], dzero[0:N, 0:16])
            nc.sync.dma_start(dpan[0:N, 112:128], dzero[0:N, 0:16])
            nc.sync.dma_start(dpan[N:NPAD, :], dzero[N:NPAD, :])
            nc.sync.dma_start(XtT[:, b * NPAD : (b + 1) * NPAD], dpan[:], transpose=True)

      # ---- gate diffusion (A resident): nch-outer, psum accumulate over mi
      # (prep/psum_pre/psum_b closed above so psum_d1 has bank room)
      with tc.tile_pool(name="psum_d1", bufs=3, space="PSUM") as psum_d1:
        for SRC, DST in ((X1, X2), (X2, X3)):
            for nch in range(NCHUNK):
                l = nlen(nch)
                ph = psum_d1.tile([128, B_LOC, CIN], F32, tag="pdiff")
                for mi in range(NCHUNK):
                    ml = nlen(mi)
                    nc.tensor.matmul(
                        ph[:l], A[:ml, mi, nch * 128 : nch * 128 + l],
                        SRC[:ml, mi, :, 0:CIN],
                        start=(mi == 0), stop=(mi == NCHUNK - 1),
                    )
                nc.scalar.activation(
                    DST[:l, nch, :, 0:CIN], ph[:l],
                    AF.Copy, scale=rinv[:l, nch : nch + 1],
                )
    # Apool/stage/xtacc closed: A + staging SBUF freed for the y phases.

    # ================= shared y-phase pools =================
    acc_pool = ctx.enter_context(tc.tile_pool(name="accp", bufs=2))
    tmp_pool = ctx.enter_context(tc.tile_pool(name="tmpp", bufs=2))
    tail_pool = ctx.enter_context(tc.tile_pool(name="tailp", bufs=2))
    ysu_pool = ctx.enter_context(tc.tile_pool(name="ysu", bufs=1))
    xtb_pool = ctx.enter_context(tc.tile_pool(name="xtb", bufs=3))

    def dred4(ysh, nblk, owid, nch, bias, n_pool=4):
        """Batched d-reduction over all 4 b: returns acc [128, B_LOC, nblk*owid].

        ysh: two half-tiles [128, B_LOC, nblk*owid*8] bf16, ysh[h] holding
        y d-slices 8h..8h+7, per-b cols [blk0 8d x owid | blk1 8d x owid].
        bias: [128, nblk*owid] (folded into the d=0 seed via STT with a
        b-broadcast (0-stride) in1 AP).
        """
        # Decomposed as 16 DVE tensor_scalar muls (4x-mode capable, unlike
        # scalar_tensor_tensor which has no DVE perf modes) + an add chain
        # split between DVE (accA: low d's, seeded with bias) and GPSIMD
        # (accB: trailing n_pool d-terms, then the accA+accB merge).
        W = nblk * owid
        accAf = acc_pool.tile([128, B_LOC, 128], BF16, tag="accA")
        accBf = acc_pool.tile([128, B_LOC, 128], BF16, tag="accB")
        accA, accB = accAf[:, :, 0:W], accBf[:, :, 0:W]
        dper = EMB // len(ysh)
        ys = [t.rearrange("p b (blk d o) -> p b blk d o", blk=nblk, d=dper)
              for t in ysh]
        bias_b = bass.AP(
            tensor=bias.tensor, offset=bias.offset,
            ap=[list(bias.ap[0]), [0, B_LOC]] + [list(d) for d in bias.ap[1:]],
        )
        t0f = tmp_pool.tile([128, B_LOC, 128], BF16, tag="t0")
        t1f = tmp_pool.tile([128, B_LOC, 128], BF16, tag="t1")
        t2f = tmp_pool.tile([128, B_LOC, 128], BF16, tag="t2")
        tt = [t0f[:, :, 0:W], t1f[:, :, 0:W], t2f[:, :, 0:W]]
        nd = EMB - n_pool
        for d in range(EMB):
            src = ys[d // dper][:, :, :, d % dper, :]
            if d == 0:
                # accA = y_0 * e_0 + bias (bias broadcast across b)
                nc.vector.scalar_tensor_tensor(
                    out=accA[:], in0=src,
                    scalar=eexpf[:, nch, 0:1],
                    in1=bias_b, op0=OP.mult, op1=OP.add,
                )
            elif d == nd:
                nc.vector.tensor_scalar(
                    out=accB[:], in0=src,
                    scalar1=eexpf[:, nch, d : d + 1], scalar2=None, op0=OP.mult,
                )
            else:
                t = tt[d % 3]
                nc.vector.tensor_scalar(
                    out=t[:], in0=src,
                    scalar1=eexpf[:, nch, d : d + 1], scalar2=None, op0=OP.mult,
                )
                eng = nc.vector if d < nd else nc.gpsimd
                eng.tensor_tensor(
                    out=accA[:] if d < nd else accB[:],
                    in0=accA[:] if d < nd else accB[:],
                    in1=t[:], op=OP.add,
                )
        nc.gpsimd.tensor_tensor(out=accA[:], in0=accA[:], in1=accB[:], op=OP.add)
        return accA

    # ================= gate y-GEMM (nch-outer, b-batched tail) =================
    with tc.tile_pool(name="ysg", bufs=2) as ysg_pool, \
         tc.tile_pool(name="psum_yg", bufs=3, space="PSUM") as psum_yg, \
         tc.tile_pool(name="psum_tg", bufs=2, space="PSUM") as psum_tg:
        def transpose_batch(srcs, nch, tag):
            # PE transposes: XkT for all 4 b; two 1-bank psum halves so the
            # y-matmul pool can take 3 bufs (6 banks)
            xtb = xtb_pool.tile([128, 3, B_LOC, 128], BF16, tag="xtb")
            xv = xtb.rearrange("p k b n -> p (k b) n")
            for h in range(2):
                pt = psum_tg.tile([128, 6 * 128], BF16, tag=f"pt{tag}")
                for j in range(6):
                    ki, b = divmod(h * 6 + j, B_LOC)
                    nc.tensor.transpose(
                        pt[:, j * 128 : (j + 1) * 128],
                        srcs[ki][:, nch, b, :], ident[:]
                    )
                nc.vector.tensor_copy(xv[:, h * 6 : (h + 1) * 6, :], pt[:])
            return xtb

        def gate_tail(nch, acc):
            acc4 = acc.rearrange("p b (blk o) -> p b blk o", blk=2)
            ztile = acc_pool.tile([128, B_LOC, DOUT], BF16, tag="ztile")
            nc.scalar.activation(ztile[:], acc4[:, :, 0, :], AF.Sigmoid)
            nc.scalar.activation(r_gate[:, nch], acc4[:, :, 1, :], AF.Sigmoid)
            # zs = z*state: stage the state slice first so the X1 write
            # never overlaps its own read range.
            zsrc = acc_pool.tile([128, B_LOC, DOUT], BF16, tag="zsrc")
            nc.gpsimd.tensor_copy(zsrc[:], X1[:, nch, :, DIN:CIN])
            nc.gpsimd.tensor_mul(X1[:, nch, :, 64:128], ztile[:], zsrc[:])

        PF = 2  # transpose prefetch distance (chunks)
        pend = []
        xtb_q = [transpose_batch((X1, X2, X3), j, "g") for j in range(PF)]
        for nch in range(NCHUNK):
            l = nlen(nch)
            if nch + PF < NCHUNK:
                xtb_q.append(transpose_batch((X1, X2, X3), nch + PF, "g"))
            xtb = xtb_q.pop(0)
            # --- y matmuls: per (b, half) psum [128, 1024]
            ysh0 = ysg_pool.tile([128, B_LOC, 1024], BF16, tag="ysg")
            ysh1 = ysg_pool.tile([128, B_LOC, 1024], BF16, tag="ysg")
            ysh = (ysh0, ysh1)
            for half in range(2):
                for b in range(B_LOC):
                    py = psum_yg.tile([128, 1024], F32, tag="pyg")
                    hs = slice(half * 512, half * 512 + 512)
                    for k in range(K):
                        nc.tensor.matmul(
                            py[:, 0:512], xtb[0:CIN, k, b, :],
                            WPg[0:CIN, k, hs],
                            start=(k == 0), stop=(k == K - 1),
                        )
                    nc.tensor.matmul(
                        py[:, 512:1024],
                        XtT[:, b * NPAD + nch * 128 : b * NPAD + nch * 128 + 128],
                        WWg[:, hs], start=True, stop=True,
                    )
                    # one of the 8 psum evicts goes to GPSIMD to unpin ACT
                    if half == 1 and b == B_LOC - 1:
                        nc.gpsimd.tensor_copy(ysh[half][:, b, :], py[:, 0:1024])
                    else:
                        nc.scalar.copy(ysh[half][:, b, :], py[:, 0:1024])
            # --- batched d-reduction; tail deferred one chunk so the next
            # chunk's PSUM evicts aren't queued behind sigmoid on ACT
            acc = dred4(ysh, 2, DOUT, nch, biasg[:, nch])
            pend.append((nch, acc))
            if len(pend) > 1:
                gate_tail(*pend.pop(0))

        while pend:
            gate_tail(*pend.pop(0))

    CAND = X1  # panels now hold [x | state(stale) | z*state]

    # ================= update diffusion =================
    C2, C3 = X2, X3
    with tc.tile_pool(name="psum_d2", bufs=3, space="PSUM") as psum_d2:
        for SRC, DST in ((CAND, C2), (C2, C3)):
            for nch in range(NCHUNK):
                l = nlen(nch)
                ph = psum_d2.tile([128, B_LOC, DOUT], F32, tag="pdiff2")
                for mi in range(NCHUNK):
                    ml = nlen(mi)
                    nc.tensor.matmul(
                        ph[:l], A[:ml, mi, nch * 128 : nch * 128 + l],
                        SRC[:ml, mi, :, 64:128],
                        start=(mi == 0), stop=(mi == NCHUNK - 1),
                    )
                nc.scalar.activation(
                    DST[:l, nch, :, 64:128], ph[:l],
                    AF.Copy, scale=rinv[:l, nch : nch + 1],
                )

    # ================= update y-GEMM + output =================
    with tc.tile_pool(name="psum_yu", bufs=3, space="PSUM") as psum_yu, \
         tc.tile_pool(name="psum_tu", bufs=2, space="PSUM") as psum_tu:
        def transpose_batch_u(nch):
            srcs = (CAND, C2, C3)
            xtb = xtb_pool.tile([128, 3, B_LOC, 128], BF16, tag="xtb")
            xv = xtb.rearrange("p k b n -> p (k b) n")
            for h in range(2):
                pt = psum_tu.tile([128, 6 * 128], BF16, tag="ptu")
                for j in range(6):
                    ki, b = divmod(h * 6 + j, B_LOC)
                    nc.tensor.transpose(
                        pt[:, j * 128 : (j + 1) * 128],
                        srcs[ki][:, nch, b, :], ident[:]
                    )
                nc.vector.tensor_copy(xv[:, h * 6 : (h + 1) * 6, :], pt[:])
            return xtb

        def upd_tail(nch, accu):
            l = nlen(nch)
            hc = tail_pool.tile([128, B_LOC, DOUT], F32, tag="hc")
            nc.scalar.activation(hc[:], accu[:], AF.Tanh)
            stf = tail_pool.tile([128, B_LOC, DOUT], F32, tag="stf")
            if l < 128:
                nc.vector.memset(stf[64:], 0.0)
            for b in range(B_LOC):
                nc.sync.dma_start(stf[:l, b], state[b, nch * 128 : nch * 128 + l, :])
            tmp = tail_pool.tile([128, B_LOC, DOUT], F32, tag="tmp")
            nc.gpsimd.tensor_sub(tmp[:], stf[:], hc[:])
            nc.gpsimd.tensor_mul(tmp[:], tmp[:], r_gate[:, nch])
            nc.gpsimd.tensor_add(tmp[:], tmp[:], hc[:])
            for b in range(B_LOC):
                nc.sync.dma_start(
                    out[b, nch * 128 : nch * 128 + l, :], tmp[:l, b, :]
                )

        PF = 2
        pend = []
        xtb_q = [transpose_batch_u(j) for j in range(PF)]
        for nch in range(NCHUNK):
            l = nlen(nch)
            if nch + PF < NCHUNK:
                xtb_q.append(transpose_batch_u(nch + PF))
            xtb = xtb_q.pop(0)
            ysu = ysu_pool.tile([128, B_LOC, 1024], BF16, tag="ysu")
            for b in range(B_LOC):
                pu = psum_yu.tile([128, 1024], F32, tag="pyu")
                for k in range(K):
                    nc.tensor.matmul(
                        pu[:, 0:512], xtb[:, k, b, :], WPu[:, k, :],
                        start=(k == 0), stop=(k == K - 1),
                    )
                nc.tensor.matmul(
                    pu[:, 512:1024],
                    XtT[:, b * NPAD + nch * 128 : b * NPAD + nch * 128 + 128],
                    WWu[:], start=True, stop=True,
                )
                nc.scalar.copy(ysu[:, b, :], pu[:, 0:1024])
            accu = dred4([ysu], 2, 32, nch, biasu[:, nch], n_pool=6)
            pend.append((nch, accu))
            if len(pend) > 1:
                upd_tail(*pend.pop(0))
        while pend:
            upd_tail(*pend.pop(0))


def make_io(nc):
    io = {}
    io["x"] = nc.dram_tensor("x", [B_LOC, N, DIN], F32, kind="ExternalInput")
    io["state"] = nc.dram_tensor("state", [B_LOC, N, DOUT], F32, kind="ExternalInput")
    io["x_full"] = nc.dram_tensor("x_full", [B_LOC, WLEN, N, CW], F32, kind="ExternalInput")
    io["node_embeddings"] = nc.dram_tensor("node_embeddings", [N, EMB], F32, kind="ExternalInput")
    io["gw_pool"] = nc.dram_tensor("gw_pool", [EMB, K, CIN, 64], F32, kind="ExternalInput")
    io["gw_win"] = nc.dram_tensor("gw_win", [EMB, CW, 64], F32, kind="ExternalInput")
    io["gb_pool"] = nc.dram_tensor("gb_pool", [EMB, 2 * DOUT], F32, kind="ExternalInput")
    io["gT"] = nc.dram_tensor("gT", [WLEN], F32, kind="ExternalInput")
    io["uw_pool"] = nc.dram_tensor("uw_pool", [EMB, K, CIN, 32], F32, kind="ExternalInput")
    io["uw_win"] = nc.dram_tensor("uw_win", [EMB, CW, 32], F32, kind="ExternalInput")
    io["ub_pool"] = nc.dram_tensor("ub_pool", [EMB, DOUT], F32, kind="ExternalInput")
    io["uT"] = nc.dram_tensor("uT", [WLEN], F32, kind="ExternalInput")
    io["out"] = nc.dram_tensor("out", [B_LOC, N, DOUT], F32, kind="ExternalOutput")
    return io


def build_module(debug=False):
    from concourse import bacc

    nc = bacc.Bacc("TRN2", target_bir_lowering=False, debug=debug)
    io = make_io(nc)
    with tile.TileContext(nc) as tc:
        build(tc, io)
    nc.finalize()
    return nc


# ======================= harness wrapper =======================
import numpy as _np

N_CORES = 8
_CACHE = {}


def _get_module():
    if "nc" not in _CACHE:
        _CACHE["nc"] = build_module()
    return _CACHE["nc"]


def make_in_maps(inputs):
    xb = _np.ascontiguousarray(inputs["x"], dtype=_np.float32)
    sb = _np.ascontiguousarray(inputs["state"], dtype=_np.float32)
    xf = _np.ascontiguousarray(inputs["x_full"], dtype=_np.float32)
    rep = {
        k: _np.ascontiguousarray(inputs[k], dtype=_np.float32)
        for k in ("node_embeddings", "gw_pool", "gw_win", "gb_pool", "gT",
                  "uw_pool", "uw_win", "ub_pool", "uT")
    }
    in_maps = []
    for i in range(N_CORES):
        m = dict(rep)
        m["x"] = xb[i * B_LOC : (i + 1) * B_LOC]
        m["state"] = sb[i * B_LOC : (i + 1) * B_LOC]
        m["x_full"] = xf[i * B_LOC : (i + 1) * B_LOC]
        in_maps.append(m)
    return in_maps


def kernel(**inputs):
    """Full-input entry point: shards over batch across 8 NeuronCores."""
    nc = _get_module()
    from concourse.bass_utils import run_bass_kernel_spmd

    in_maps = make_in_maps(inputs)
    res = run_bass_kernel_spmd(nc, in_maps, core_ids=list(range(N_CORES)))
    return _np.concatenate([res.results[i]["out"] for i in range(N_CORES)], axis=0)



# revision 17
# speedup vs baseline: 1.0311x; 1.0199x over previous
import sys
for _p in ('/opt/trn_rl_repo',):
    if _p not in sys.path:
        sys.path.insert(0, _p)

"""NLSGCRN cell Bass/Tile kernel for TRN2, batch-sharded SPMD over 8 cores.

Per-core shapes (b_loc = 4 batches):
  x [4,2000,32], state [4,2000,64], x_full [4,12,2000,48], emb [2000,16],
  pools gw/uw/gb/ub/gT/uT, out h [4,2000,64].

v2 structure (vs v1 baseline):
- y-phases loop nch-outer / b-inner: the per-node d-reduction batches all
  4 batches into one op (the e[n,d] scalar is per-partition and shared
  across b), split DVE (d < D_DVE, seeded with bias) / GPSIMD (rest).
- XkT stationaries via PE transposes (identity matmul) into bf16 PSUM,
  evicted by DVE, replacing v1's DRAM-bounce DMA transposes.
- A is spilled to DRAM at generation and its SBUF tile is freed after the
  gate hops; update diffusion streams A chunks back mi-outer with all 16
  output-chunk accumulators resident in PSUM (8 banks exactly).
- PSUM bank budget in y phases: y tiles [128,1024] f32 (2 banks) x2 bufs
  + transpose batch [128,1536] bf16 (2 banks) x2 bufs.
"""

from contextlib import ExitStack

import concourse.bass as bass
import concourse.tile as tile
from concourse import mybir
from concourse._compat import with_exitstack
from concourse.masks import make_identity

F32 = mybir.dt.float32
F32R = mybir.dt.float32r
BF16 = mybir.dt.bfloat16
AF = mybir.ActivationFunctionType
OP = mybir.AluOpType

B_LOC = 4
N = 2000
NCHUNK = 16           # ceil(2000/128)
NFULL = (NCHUNK - 1) * 128   # 1920
NPAD = NCHUNK * 128   # 2048
DIN, DOUT = 32, 64
CIN = 96
CW = 48
WLEN = 12
EMB = 16
K = 3

D_DVE = 8   # d-reduction: first D_DVE iterations on DVE, rest on GPSIMD


def nlen(nch):
    return 128 if nch < NCHUNK - 1 else N - NFULL  # last = 80


def chunked_load(nc, dst, src, eng=None):
    """dst [128, NCHUNK, ...inner] <- src [2000, ...inner] splitting rows."""
    eng = eng or nc.sync
    inner = src.shape[1:]
    eng.dma_start(
        dst[:, 0 : NCHUNK - 1],
        src[0:NFULL].rearrange(
            "(c p) " + " ".join(f"i{j}" for j in range(len(inner)))
            + " -> p c " + " ".join(f"i{j}" for j in range(len(inner))),
            p=128,
        ),
    )
    eng.dma_start(dst[0 : N - NFULL, NCHUNK - 1], src[NFULL:N])


@with_exitstack
def build(ctx: ExitStack, tc: tile.TileContext, io: dict):
    nc = tc.nc

    io = {k: (v[:] if not isinstance(v, bass.AP) else v) for k, v in io.items()}
    x, state, x_full = io["x"], io["state"], io["x_full"]
    emb = io["node_embeddings"]
    out = io["out"]

    const = ctx.enter_context(tc.tile_pool(name="const", bufs=1))
    big = ctx.enter_context(tc.tile_pool(name="big", bufs=1))
    dram = ctx.enter_context(tc.tile_pool(name="dram", bufs=6, space="DRAM"))

    # ================= constants / weights =================
    eexpf = const.tile([128, NCHUNK, EMB], F32)
    nc.vector.memset(eexpf[:], 0.0)
    chunked_load(nc, eexpf, emb, eng=nc.gpsimd)

    ident = const.tile([128, 128], BF16)
    make_identity(nc, ident)

    biasg = const.tile([128, NCHUNK, 2 * DOUT], BF16)
    biasu = const.tile([128, NCHUNK, DOUT], BF16)
    nc.vector.memset(biasg[64:, NCHUNK - 1], 0.0)
    nc.vector.memset(biasu[64:, NCHUNK - 1], 0.0)
    rinv = const.tile([128, NCHUNK], F32)
    dsum_all = const.tile([128, NCHUNK], F32)

    # WPg [128, 3, 1024] bf16: rows 0:96 = c, cols (d,o) d-major.
    WPg = const.tile([128, K, EMB * 64], BF16)
    nc.vector.memset(WPg[:], 0.0)
    WPu = const.tile([128, K, EMB * 32], BF16)
    nc.vector.memset(WPu[:], 0.0)
    WWg = const.tile([128, EMB * 64], BF16)
    nc.vector.memset(WWg[:], 0.0)
    WWu = const.tile([128, EMB * 32], BF16)
    nc.vector.memset(WWu[:], 0.0)

    Tb = const.tile([128, 2, WLEN], F32)
    for w, name in ((0, "gT"), (1, "uT")):
        src = io[name][:]
        nc.sync.dma_start(
            Tb[:, w, :],
            bass.AP(tensor=src.tensor, offset=src.offset, ap=[[0, 128]] + list(src.ap)),
        )

    X1 = big.tile([128, NCHUNK, B_LOC, 128], BF16, tag="slot1")
    X2 = big.tile([128, NCHUNK, B_LOC, 128], BF16, tag="slot2")
    X3 = big.tile([128, NCHUNK, B_LOC, 128], BF16, tag="slot3")
    nc.vector.memset(X1[:], 0.0)
    nc.gpsimd.memset(X2[:], 0.0)
    nc.gpsimd.memset(X3[:], 0.0)
    XtT = big.tile([128, B_LOC * NPAD], BF16, tag="XtT")
    r_gate = big.tile([128, NCHUNK, B_LOC, DOUT], BF16, tag="rgate")
    dA = dram.tile([128, NCHUNK * N], BF16, tag="dA")

    FLAT = N * CW // 128  # 750

    with tc.tile_pool(name="stage", bufs=3) as stage, \
         tc.tile_pool(name="stage3", bufs=3) as stage3, \
         tc.tile_pool(name="xtacc", bufs=1) as xtacc, \
         tc.tile_pool(name="apool", bufs=1) as apool:
      A = apool.tile([128, NCHUNK, N], BF16, tag="A")
      with tc.tile_pool(name="prep", bufs=1) as prep, \
           tc.tile_pool(name="psum_pre", bufs=1, space="PSUM") as psum_pre, \
           tc.tile_pool(name="psum_b", bufs=2, space="PSUM") as psum_b:
        # ---- small loads all go through the gpsimd SWDGE queue (which can
        # also cast f32->bf16 in flight); nc.sync's HWDGE queue is reserved
        # for the x_full stream + XtT bounce.
        embT_raw = prep.tile([EMB, N], F32)
        nc.sync.dma_start(embT_raw[:], emb.rearrange("n d -> d n"))
        embT = prep.tile([EMB, N], F32R)
        nc.vector.tensor_copy(embT[:], embT_raw[:])
        gbp_raw = prep.tile([EMB, 2 * DOUT], F32)
        nc.gpsimd.dma_start(gbp_raw[:], io["gb_pool"][:])
        gbp_s = prep.tile([EMB, 2 * DOUT], F32R)
        nc.vector.tensor_copy(gbp_s[:], gbp_raw[:])
        ubp_raw = prep.tile([EMB, DOUT], F32)
        nc.gpsimd.dma_start(ubp_raw[:], io["ub_pool"][:])
        ubp_s = prep.tile([EMB, DOUT], F32R)
        nc.vector.tensor_copy(ubp_s[:], ubp_raw[:])

        # ---- x/state -> X1 [x | state] bf16 via casting SWDGE DMAs
        for b in range(B_LOC):
            chunked_load(nc, X1[:, :, b, 0:DIN], x[b], eng=nc.gpsimd)
            chunked_load(nc, X1[:, :, b, DIN:CIN], state[b], eng=nc.gpsimd)

        # ---- weight pools: casting SWDGE DMAs straight into packed tiles
        for k in range(K):
            nc.gpsimd.dma_start(
                WPg[0:CIN, k].rearrange("p (d o) -> p d o", d=EMB),
                io["gw_pool"][:, k].rearrange("d c o -> c d o"),
            )
            nc.gpsimd.dma_start(
                WPu[0:32, k].rearrange("p (d o) -> p d o", d=EMB),
                io["uw_pool"][:, k, 0:32, :].rearrange("d c o -> c d o"),
            )
            nc.gpsimd.dma_start(
                WPu[64:128, k].rearrange("p (d o) -> p d o", d=EMB),
                io["uw_pool"][:, k, 32:96, :].rearrange("d c o -> c d o"),
            )
        nc.gpsimd.dma_start(
            WWg[0:CW].rearrange("p (d o) -> p d o", d=EMB),
            io["gw_win"].rearrange("d i o -> i d o"),
        )
        # rows 64:112 (matches packed XtT where xt_u.T sits at partitions 64:112)
        nc.gpsimd.dma_start(
            WWu[64 : 64 + CW].rearrange("p (d o) -> p d o", d=EMB),
            io["uw_win"].rearrange("d i o -> i d o"),
        )

        def agen_chunk(nch):
            l = nlen(nch)
            nsl = slice(nch * 128, nch * 128 + l)
            pg = psum_pre.tile([128, N], F32, tag="pg")
            for mj in range(4):
                m0 = mj * 512
                mw = min(512, N - m0)
                nc.tensor.matmul(
                    pg[:l, m0 : m0 + mw], embT[:, nsl],
                    embT[:, m0 : m0 + mw], start=True, stop=True,
                )
            nc.scalar.activation(A[:l, nch, :], pg[:l, :], AF.Exp)
            nc.vector.tensor_scalar(
                out=A[:l, nch, :], in0=A[:l, nch, :],
                scalar1=1.0, scalar2=0.0, op0=OP.max, op1=OP.add,
                accum_out=dsum_all[:l, nch : nch + 1],
            )
            nc.vector.reciprocal(rinv[:l, nch : nch + 1], dsum_all[:l, nch : nch + 1])

        def bias_chunk(nch):
            l = nlen(nch)
            nsl = slice(nch * 128, nch * 128 + l)
            pb = psum_b.tile([128, 3 * DOUT], F32, tag="pbias")
            nc.tensor.matmul(
                pb[:l, 0 : 2 * DOUT], embT[:, nsl], gbp_s[:], start=True, stop=True,
            )
            nc.tensor.matmul(
                pb[:l, 2 * DOUT :], embT[:, nsl], ubp_s[:], start=True, stop=True,
            )
            nc.scalar.copy(biasg[:l, nch, :], pb[:l, 0 : 2 * DOUT])
            nc.scalar.copy(biasu[:l, nch, :], pb[:l, 2 * DOUT :])

        # ---- window t-contraction (flat layout) + DRAM-bounce into XtT,
        # with A-gen / bias chunks interleaved so their DVE/ACT/PE work
        # overlaps the x_full DMA stream.
        zeros128 = const.tile([128, 128], BF16)
        nc.vector.memset(zeros128[:], 0.0)
        dzero = dram.tile([NPAD, 128], BF16, tag="dzero")
        nc.sync.dma_start(
            dzero.rearrange("(c p) o -> p c o", p=128),
            bass.AP(tensor=zeros128.tensor, offset=zeros128.offset,
                    ap=[[1, 128], [0, NCHUNK], [1, 128]]),
        )
        xt_g = xtacc.tile([128, B_LOC, FLAT], F32)
        xt_u = xtacc.tile([128, B_LOC, FLAT], BF16)
        # b-outer so each batch's XtT panel packs as soon as its 12 t-loads
        # land, overlapping the remaining loads and the gate diffusion
        for b in range(B_LOC):
            for t in range(WLEN):
                st = stage3.tile([128, FLAT], F32, tag="xw")
                nc.sync.dma_start(
                    st[:],
                    x_full[b, t].rearrange("n i -> (n i)").rearrange("(p f) -> p f", p=128),
                )
                for w, acc in ((0, xt_g), (1, xt_u)):
                    if t == 0:
                        nc.vector.tensor_scalar(
                            out=acc[:, b, :], in0=st[:],
                            scalar1=Tb[:, w, 0:1], scalar2=None, op0=OP.mult,
                        )
                    else:
                        nc.vector.scalar_tensor_tensor(
                            out=acc[:, b, :], in0=st[:],
                            scalar=Tb[:, w, t : t + 1],
                            in1=acc[:, b, :], op0=OP.mult, op1=OP.add,
                        )
                idx = b * WLEN + t
                if idx < NCHUNK:
                    agen_chunk(idx)
                elif idx < 2 * NCHUNK:
                    bias_chunk(idx - NCHUNK)
            xgb16 = stage.tile([128, 2, FLAT], BF16, tag="stg")
            nc.gpsimd.tensor_copy(xgb16[:, 0, :], xt_g[:, b, :])
            nc.gpsimd.tensor_copy(xgb16[:, 1, :], xt_u[:, b, :])
            dflat = dram.tile([2, 128, FLAT], BF16, tag="dflat")
            nc.sync.dma_start(dflat.rearrange("w p f -> p w f"), xgb16[:])
            dpan = dram.tile([NPAD, 128], BF16, tag="pan")
            dfv = dflat.rearrange("w p f -> w (p f)").rearrange("w (n i) -> w n i", n=N)
            nc.sync.dma_start(dpan[0:N, 0:CW], dfv[0])
            nc.sync.dma_start(dpan[0:N, 64 : 64 + CW], dfv[1])
            nc.sync.dma_start(dpan[0:N, CW:64], dzero[0:N, 0:16])
            nc.sync.dma_start(dpan[0:N, 112:128], dzero[0:N, 0:16])
            nc.sync.dma_start(dpan[N:NPAD, :], dzero[N:NPAD, :])
            nc.sync.dma_start(XtT[:, b * NPAD : (b + 1) * NPAD], dpan[:], transpose=True)

      # ---- gate diffusion (A resident): nch-outer, psum accumulate over mi
      # (prep/psum_pre/psum_b closed above so psum_d1 has bank room)
      with tc.tile_pool(name="psum_d1", bufs=3, space="PSUM") as psum_d1:
        for SRC, DST in ((X1, X2), (X2, X3)):
            for nch in range(NCHUNK):
                l = nlen(nch)
                ph = psum_d1.tile([128, B_LOC, CIN], F32, tag="pdiff")
                for mi in range(NCHUNK):
                    ml = nlen(mi)
                    nc.tensor.matmul(
                        ph[:l], A[:ml, mi, nch * 128 : nch * 128 + l],
                        SRC[:ml, mi, :, 0:CIN],
                        start=(mi == 0), stop=(mi == NCHUNK - 1),
                    )
                nc.scalar.activation(
                    DST[:l, nch, :, 0:CIN], ph[:l],
                    AF.Copy, scale=rinv[:l, nch : nch + 1],
                )
      # ---- spill A to DRAM; the update diffusion streams it back mi-outer.
      nc.sync.dma_start(dA[:], A.rearrange("p c n -> p (c n)"))
    # Apool/stage/xtacc closed: A + staging SBUF freed for the y phases.

    # ================= shared y-phase pools =================
    acc_pool = ctx.enter_context(tc.tile_pool(name="accp", bufs=2))
    tmp_pool = ctx.enter_context(tc.tile_pool(name="tmpp", bufs=2))
    tail_pool = ctx.enter_context(tc.tile_pool(name="tailp", bufs=2))
    ysu_pool = ctx.enter_context(tc.tile_pool(name="ysu", bufs=2))
    xtb_pool = ctx.enter_context(tc.tile_pool(name="xtb", bufs=3))

    def dred4(ysh, nblk, owid, nch, bias, n_pool=4):
        """Batched d-reduction over all 4 b: returns acc [128, B_LOC, nblk*owid].

        ysh: two half-tiles [128, B_LOC, nblk*owid*8] bf16, ysh[h] holding
        y d-slices 8h..8h+7, per-b cols [blk0 8d x owid | blk1 8d x owid].
        bias: [128, nblk*owid] (folded into the d=0 seed via STT with a
        b-broadcast (0-stride) in1 AP).
        """
        # Decomposed as 16 DVE tensor_scalar muls (4x-mode capable, unlike
        # scalar_tensor_tensor which has no DVE perf modes) + an add chain
        # split between DVE (accA: low d's, seeded with bias) and GPSIMD
        # (accB: trailing n_pool d-terms, then the accA+accB merge).
        W = nblk * owid
        accAf = acc_pool.tile([128, B_LOC, 128], BF16, tag="accA")
        accBf = acc_pool.tile([128, B_LOC, 128], BF16, tag="accB")
        accA, accB = accAf[:, :, 0:W], accBf[:, :, 0:W]
        dper = EMB // len(ysh)
        ys = [t.rearrange("p b (blk d o) -> p b blk d o", blk=nblk, d=dper)
              for t in ysh]
        bias_b = bass.AP(
            tensor=bias.tensor, offset=bias.offset,
            ap=[list(bias.ap[0]), [0, B_LOC]] + [list(d) for d in bias.ap[1:]],
        )
        t0f = tmp_pool.tile([128, B_LOC, 128], BF16, tag="t0")
        t1f = tmp_pool.tile([128, B_LOC, 128], BF16, tag="t1")
        t2f = tmp_pool.tile([128, B_LOC, 128], BF16, tag="t2")
        tt = [t0f[:, :, 0:W], t1f[:, :, 0:W], t2f[:, :, 0:W]]
        nd = EMB - n_pool
        for d in range(EMB):
            src = ys[d // dper][:, :, :, d % dper, :]
            if d == 0:
                # accA = y_0 * e_0 + bias (bias broadcast across b)
                nc.vector.scalar_tensor_tensor(
                    out=accA[:], in0=src,
                    scalar=eexpf[:, nch, 0:1],
                    in1=bias_b, op0=OP.mult, op1=OP.add,
                )
            elif d == nd:
                nc.vector.tensor_scalar(
                    out=accB[:], in0=src,
                    scalar1=eexpf[:, nch, d : d + 1], scalar2=None, op0=OP.mult,
                )
            else:
                t = tt[d % 3]
                nc.vector.tensor_scalar(
                    out=t[:], in0=src,
                    scalar1=eexpf[:, nch, d : d + 1], scalar2=None, op0=OP.mult,
                )
                eng = nc.vector if d < nd else nc.gpsimd
                eng.tensor_tensor(
                    out=accA[:] if d < nd else accB[:],
                    in0=accA[:] if d < nd else accB[:],
                    in1=t[:], op=OP.add,
                )
        nc.gpsimd.tensor_tensor(out=accA[:], in0=accA[:], in1=accB[:], op=OP.add)
        return accA

    # ================= gate y-GEMM (nch-outer, b-batched tail) =================
    with tc.tile_pool(name="ysg", bufs=2) as ysg_pool, \
         tc.tile_pool(name="psum_yg", bufs=3, space="PSUM") as psum_yg, \
         tc.tile_pool(name="psum_tg", bufs=2, space="PSUM") as psum_tg:
        def transpose_batch(srcs, nch, tag):
            # PE transposes: XkT for all 4 b; two 1-bank psum halves so the
            # y-matmul pool can take 3 bufs (6 banks)
            xtb = xtb_pool.tile([128, 3, B_LOC, 128], BF16, tag="xtb")
            xv = xtb.rearrange("p k b n -> p (k b) n")
            for h in range(2):
                pt = psum_tg.tile([128, 6 * 128], BF16, tag=f"pt{tag}")
                for j in range(6):
                    ki, b = divmod(h * 6 + j, B_LOC)
                    nc.tensor.transpose(
                        pt[:, j * 128 : (j + 1) * 128],
                        srcs[ki][:, nch, b, :], ident[:]
                    )
                nc.vector.tensor_copy(xv[:, h * 6 : (h + 1) * 6, :], pt[:])
            return xtb

        def gate_tail(nch, acc):
            acc4 = acc.rearrange("p b (blk o) -> p b blk o", blk=2)
            ztile = acc_pool.tile([128, B_LOC, DOUT], BF16, tag="ztile")
            nc.scalar.activation(ztile[:], acc4[:, :, 0, :], AF.Sigmoid)
            nc.scalar.activation(r_gate[:, nch], acc4[:, :, 1, :], AF.Sigmoid)
            # zs = z*state: stage the state slice first so the X1 write
            # never overlaps its own read range.
            zsrc = acc_pool.tile([128, B_LOC, DOUT], BF16, tag="zsrc")
            nc.gpsimd.tensor_copy(zsrc[:], X1[:, nch, :, DIN:CIN])
            nc.gpsimd.tensor_mul(X1[:, nch, :, 64:128], ztile[:], zsrc[:])

        PF = 2  # transpose prefetch distance (chunks)
        pend = []
        xtb_q = [transpose_batch((X1, X2, X3), j, "g") for j in range(PF)]
        for nch in range(NCHUNK):
            l = nlen(nch)
            if nch + PF < NCHUNK:
                xtb_q.append(transpose_batch((X1, X2, X3), nch + PF, "g"))
            xtb = xtb_q.pop(0)
            # --- y matmuls: per (b, half) psum [128, 1024]
            ysh0 = ysg_pool.tile([128, B_LOC, 1024], BF16, tag="ysg")
            ysh1 = ysg_pool.tile([128, B_LOC, 1024], BF16, tag="ysg")
            ysh = (ysh0, ysh1)
            for half in range(2):
                for b in range(B_LOC):
                    py = psum_yg.tile([128, 1024], F32, tag="pyg")
                    hs = slice(half * 512, half * 512 + 512)
                    for k in range(K):
                        nc.tensor.matmul(
                            py[:, 0:512], xtb[0:CIN, k, b, :],
                            WPg[0:CIN, k, hs],
                            start=(k == 0), stop=(k == K - 1),
                        )
                    nc.tensor.matmul(
                        py[:, 512:1024],
                        XtT[:, b * NPAD + nch * 128 : b * NPAD + nch * 128 + 128],
                        WWg[:, hs], start=True, stop=True,
                    )
                    # one of the 8 psum evicts goes to GPSIMD to unpin ACT
                    if half == 1 and b == B_LOC - 1:
                        nc.gpsimd.tensor_copy(ysh[half][:, b, :], py[:, 0:1024])
                    else:
                        nc.scalar.copy(ysh[half][:, b, :], py[:, 0:1024])
            # --- batched d-reduction; tail deferred one chunk so the next
            # chunk's PSUM evicts aren't queued behind sigmoid on ACT
            acc = dred4(ysh, 2, DOUT, nch, biasg[:, nch])
            pend.append((nch, acc))
            if len(pend) > 1:
                gate_tail(*pend.pop(0))

        while pend:
            gate_tail(*pend.pop(0))

    CAND = X1  # panels now hold [x | state(stale) | z*state]

    # ================= update diffusion =================
    # mi-outer with all 16 out-chunk accumulators psum-resident (8 banks
    # exactly); A chunks stream back from DRAM through a small rotating pool,
    # so A's 62.5KB/partition never coexists with the y-phase tiles.
    C2, C3 = X2, X3
    dAv = dA.rearrange("p (c n) -> p c n", n=N)
    with tc.tile_pool(name="apool2", bufs=3) as apool2:
        for SRC, DST in ((CAND, C2), (C2, C3)):
            with tc.tile_pool(name="psum_ud", bufs=1, space="PSUM") as psum_ud:
                accs = psum_ud.tile([128, NCHUNK, B_LOC, DOUT], F32)
                for mi in range(NCHUNK):
                    ml = nlen(mi)
                    Ai = apool2.tile([128, N], BF16, tag="Ai")
                    nc.sync.dma_start(Ai[:ml], dAv[:ml, mi, :])
                    for nch in range(NCHUNK):
                        l = nlen(nch)
                        nc.tensor.matmul(
                            accs[:l, nch], Ai[:ml, nch * 128 : nch * 128 + l],
                            SRC[:ml, mi, :, 64:128],
                            start=(mi == 0), stop=(mi == NCHUNK - 1),
                        )
                for nch in range(NCHUNK):
                    l = nlen(nch)
                    nc.scalar.activation(
                        DST[:l, nch, :, 64:128], accs[:l, nch],
                        AF.Copy, scale=rinv[:l, nch : nch + 1],
                    )

    # ================= update y-GEMM + output =================
    with tc.tile_pool(name="psum_yu", bufs=3, space="PSUM") as psum_yu, \
         tc.tile_pool(name="psum_tu", bufs=2, space="PSUM") as psum_tu:
        def transpose_batch_u(nch):
            srcs = (CAND, C2, C3)
            xtb = xtb_pool.tile([128, 3, B_LOC, 128], BF16, tag="xtb")
            xv = xtb.rearrange("p k b n -> p (k b) n")
            for h in range(2):
                pt = psum_tu.tile([128, 6 * 128], BF16, tag="ptu")
                for j in range(6):
                    ki, b = divmod(h * 6 + j, B_LOC)
                    nc.tensor.transpose(
                        pt[:, j * 128 : (j + 1) * 128],
                        srcs[ki][:, nch, b, :], ident[:]
                    )
                nc.vector.tensor_copy(xv[:, h * 6 : (h + 1) * 6, :], pt[:])
            return xtb

        def upd_tail(nch, accu):
            l = nlen(nch)
            hc = tail_pool.tile([128, B_LOC, DOUT], F32, tag="hc")
            nc.scalar.activation(hc[:], accu[:], AF.Tanh)
            stf = tail_pool.tile([128, B_LOC, DOUT], F32, tag="stf")
            if l < 128:
                nc.vector.memset(stf[64:], 0.0)
            for b in range(B_LOC):
                nc.sync.dma_start(stf[:l, b], state[b, nch * 128 : nch * 128 + l, :])
            tmp = tail_pool.tile([128, B_LOC, DOUT], F32, tag="tmp")
            nc.gpsimd.tensor_sub(tmp[:], stf[:], hc[:])
            nc.gpsimd.tensor_mul(tmp[:], tmp[:], r_gate[:, nch])
            nc.gpsimd.tensor_add(tmp[:], tmp[:], hc[:])
            for b in range(B_LOC):
                nc.sync.dma_start(
                    out[b, nch * 128 : nch * 128 + l, :], tmp[:l, b, :]
                )

        PF = 2
        pend = []
        xtb_q = [transpose_batch_u(j) for j in range(PF)]
        for nch in range(NCHUNK):
            l = nlen(nch)
            if nch + PF < NCHUNK:
                xtb_q.append(transpose_batch_u(nch + PF))
            xtb = xtb_q.pop(0)
            ysu = ysu_pool.tile([128, B_LOC, 1024], BF16, tag="ysu")
            for b in range(B_LOC):
                pu = psum_yu.tile([128, 1024], F32, tag="pyu")
                for k in range(K):
                    nc.tensor.matmul(
                        pu[:, 0:512], xtb[:, k, b, :], WPu[:, k, :],
                        start=(k == 0), stop=(k == K - 1),
                    )
                nc.tensor.matmul(
                    pu[:, 512:1024],
                    XtT[:, b * NPAD + nch * 128 : b * NPAD + nch * 128 + 128],
                    WWu[:], start=True, stop=True,
                )
                nc.scalar.copy(ysu[:, b, :], pu[:, 0:1024])
            accu = dred4([ysu], 2, 32, nch, biasu[:, nch], n_pool=6)
            pend.append((nch, accu))
            if len(pend) > 1:
                upd_tail(*pend.pop(0))
        while pend:
            upd_tail(*pend.pop(0))


def make_io(nc):
    io = {}
    io["x"] = nc.dram_tensor("x", [B_LOC, N, DIN], F32, kind="ExternalInput")
    io["state"] = nc.dram_tensor("state", [B_LOC, N, DOUT], F32, kind="ExternalInput")
    io["x_full"] = nc.dram_tensor("x_full", [B_LOC, WLEN, N, CW], F32, kind="ExternalInput")
    io["node_embeddings"] = nc.dram_tensor("node_embeddings", [N, EMB], F32, kind="ExternalInput")
    io["gw_pool"] = nc.dram_tensor("gw_pool", [EMB, K, CIN, 64], F32, kind="ExternalInput")
    io["gw_win"] = nc.dram_tensor("gw_win", [EMB, CW, 64], F32, kind="ExternalInput")
    io["gb_pool"] = nc.dram_tensor("gb_pool", [EMB, 2 * DOUT], F32, kind="ExternalInput")
    io["gT"] = nc.dram_tensor("gT", [WLEN], F32, kind="ExternalInput")
    io["uw_pool"] = nc.dram_tensor("uw_pool", [EMB, K, CIN, 32], F32, kind="ExternalInput")
    io["uw_win"] = nc.dram_tensor("uw_win", [EMB, CW, 32], F32, kind="ExternalInput")
    io["ub_pool"] = nc.dram_tensor("ub_pool", [EMB, DOUT], F32, kind="ExternalInput")
    io["uT"] = nc.dram_tensor("uT", [WLEN], F32, kind="ExternalInput")
    io["out"] = nc.dram_tensor("out", [B_LOC, N, DOUT], F32, kind="ExternalOutput")
    return io


def build_module(debug=False):
    from concourse import bacc

    nc = bacc.Bacc("TRN2", target_bir_lowering=False, debug=debug)
    io = make_io(nc)
    with tile.TileContext(nc) as tc:
        build(tc, io)
    nc.finalize()
    return nc


# ======================= harness wrapper =======================
import numpy as _np

N_CORES = 8
_CACHE = {}


def _get_module():
    if "nc" not in _CACHE:
        _CACHE["nc"] = build_module()
    return _CACHE["nc"]


def make_in_maps(inputs):
    xb = _np.ascontiguousarray(inputs["x"], dtype=_np.float32)
    sb = _np.ascontiguousarray(inputs["state"], dtype=_np.float32)
    xf = _np.ascontiguousarray(inputs["x_full"], dtype=_np.float32)
    rep = {
        k: _np.ascontiguousarray(inputs[k], dtype=_np.float32)
        for k in ("node_embeddings", "gw_pool", "gw_win", "gb_pool", "gT",
                  "uw_pool", "uw_win", "ub_pool", "uT")
    }
    in_maps = []
    for i in range(N_CORES):
        m = dict(rep)
        m["x"] = xb[i * B_LOC : (i + 1) * B_LOC]
        m["state"] = sb[i * B_LOC : (i + 1) * B_LOC]
        m["x_full"] = xf[i * B_LOC : (i + 1) * B_LOC]
        in_maps.append(m)
    return in_maps


def kernel(**inputs):
    """Full-input entry point: shards over batch across 8 NeuronCores."""
    nc = _get_module()
    from concourse.bass_utils import run_bass_kernel_spmd

    in_maps = make_in_maps(inputs)
    res = run_bass_kernel_spmd(nc, in_maps, core_ids=list(range(N_CORES)))
    return _np.concatenate([res.results[i]["out"] for i in range(N_CORES)], axis=0)

